# revision 2
# baseline (speedup 1.0000x reference)
"""AntiBurst kernel for Trainium2, data-parallel over batch on 8 NeuronCores.

Reference computation (per batch b):
    xf  = x[b].reshape(D, N)                      # D=768, N=1024
    G   = xf.T @ xf                               # (N, N)
    dis = (-2 + 2*G) * ab_w + ab_b = G*s + t      # s = 2*ab_w, t = ab_b - 2*ab_w
    w   = sum_m sigmoid(dis[n, m])                # (N,)
    out = xf / w**ab_p  =  xf * exp(-ab_p * ln w)

Device strategy per core (4 batches/core):
  - Gram matrix via TensorE in bf16 (inputs pre-cast on host), f32 PSUM accum.
  - sigmoid + row-sum fused on ScalarE (activation with accum_out), scale/bias
    taken from runtime ab_params broadcast to [128,1] columns.
  - w^-p via Ln then Exp(scale=-p) on ScalarE.
  - r=w^-p transposed via TensorE, bounced through DRAM, broadcast-DMA'd to
    all 128 partitions, then multiplied into x on VectorE.
"""

import numpy as np
import ml_dtypes

import concourse.bass as bass
import concourse.mybir as mybir
import concourse.tile as tile
from concourse import bacc
from concourse.bass_utils import run_bass_kernel_spmd
from concourse.masks import make_identity

B, D, H, W = 32, 768, 32, 32
N = H * W                      # 1024
NCORES = 8
BPC = B // NCORES              # batches per core = 4
DC = D // 128                  # d-chunks = 6
NJ = N // 128                  # n row-blocks = 8
F32 = mybir.dt.float32
BF16 = mybir.dt.bfloat16


def build(iters: int = 1):
    nc = bacc.Bacc("TRN2", target_bir_lowering=False, debug=False, num_devices=NCORES)
    x_d = nc.dram_tensor("x", [BPC, D, N], BF16, kind="ExternalInput").ap()
    ab_d = nc.dram_tensor("ab_params", [1, 3], F32, kind="ExternalInput").ap()
    out_d = nc.dram_tensor("out", [BPC, D, N], BF16, kind="ExternalOutput").ap()

    with tile.TileContext(nc) as tc:
        with (
            tc.tile_pool(name="const", bufs=1) as constp,
            tc.tile_pool(name="x", bufs=3) as xpool,
            tc.tile_pool(name="sig", bufs=2) as sigpool,
            tc.tile_pool(name="wcol", bufs=2) as wpool,
            tc.tile_pool(name="rbc", bufs=2) as rbcpool,
            tc.tile_pool(name="ob", bufs=2) as opool,
            tc.tile_pool(name="psg", bufs=3, space="PSUM") as psgpool,
            tc.tile_pool(name="pst", bufs=1, space="PSUM") as pstpool,
            tc.tile_pool(name="dram", bufs=2, space="DRAM") as drampool,
        ):
            # --- runtime scalars ---
            ab_sb = constp.tile([128, 3], F32)
            nc.sync.dma_start(ab_sb[:], ab_d.to_broadcast((128, 3)))
            s_col = constp.tile([128, 1], F32)
            nc.vector.tensor_scalar_mul(s_col[:], ab_sb[:, 0:1], 2.0)
            t_col = constp.tile([128, 1], F32)
            nc.vector.tensor_sub(t_col[:], ab_sb[:, 1:2], s_col[:])
            negp = constp.tile([128, 1], F32)
            nc.vector.tensor_scalar_mul(negp[:], ab_sb[:, 2:3], -1.0)
            ident = constp.tile([128, 128], F32)
            make_identity(nc, ident[:])

            for _ in range(iters):
                for b in range(BPC):
                    # one big strided DMA: x[b] (768,1024) -> [128, 6*1024]
                    xbf = xpool.tile([128, DC, N], BF16)
                    nc.sync.dma_start(
                        xbf[:],
                        x_d[b].rearrange("(c p) n -> p c n", p=128),
                    )

                    w_sb = wpool.tile([128, NJ], F32, tag="w")
                    for j in range(NJ):
                        psG = psgpool.tile([128, N], F32)
                        for c in range(DC):
                            lhsT = xbf[:, c, j * 128:(j + 1) * 128]
                            for h in range(2):
                                nc.tensor.matmul(
                                    psG[:, h * 512:(h + 1) * 512],
                                    lhsT,
                                    xbf[:, c, h * 512:(h + 1) * 512],
                                    start=(c == 0),
                                    stop=(c == DC - 1),
                                )
                        sg = sigpool.tile([128, N], BF16)
                        nc.scalar.activation(
                            sg[:],
                            psG[:],
                            mybir.ActivationFunctionType.Sigmoid,
                            bias=t_col[:],
                            scale=s_col[:],
                            accum_out=w_sb[:, j:j + 1],
                        )

                    # r = exp(-p * ln(w)), still [128, 8] layout
                    lnw = wpool.tile([128, NJ], F32, tag="lnw")
                    nc.scalar.activation(
                        lnw[:], w_sb[:], mybir.ActivationFunctionType.Ln
                    )
                    r_sb = wpool.tile([128, NJ], F32, tag="r")
                    nc.scalar.activation(
                        r_sb[:], lnw[:], mybir.ActivationFunctionType.Exp,
                        scale=negp[:],
                    )

                    # transpose r to row-major n order: [8, 128]
                    ps_t = pstpool.tile([NJ, 128], F32)
                    nc.tensor.transpose(ps_t[:], r_sb[:], ident[:])
                    r_t = wpool.tile([NJ, 128], F32, tag="rt")
                    nc.vector.tensor_copy(r_t[:], ps_t[:])

                    # bounce through DRAM, then broadcast to 128 partitions
                    r_dram = drampool.tile([1, N], F32)
                    nc.sync.dma_start(
                        r_dram[:].rearrange("o (j p) -> (o j) p", p=128), r_t[:]
                    )
                    r_bc = rbcpool.tile([128, N], F32)
                    nc.sync.dma_start(r_bc[:], r_dram[:].to_broadcast((128, N)))

                    # out = x * r  (bf16), one big strided DMA back
                    obf = opool.tile([128, DC, N], BF16)
                    for c in range(DC):
                        nc.vector.tensor_mul(obf[:, c, :], xbf[:, c, :], r_bc[:])
                    nc.sync.dma_start(
                        out_d[b].rearrange("(c p) n -> p c n", p=128),
                        obf[:],
                    )
    nc.compile()
    return nc


_CACHE: dict[int, object] = {}


def _get_nc(iters: int = 1):
    if iters not in _CACHE:
        _CACHE[iters] = build(iters)
    return _CACHE[iters]


def run_sharded(xbf_shards, ab2d, iters: int = 1):
    """xbf_shards: list of 8 arrays [BPC, D, N] bf16; returns list of out shards + result obj."""
    nc = _get_nc(iters)
    in_maps = [{"x": xbf_shards[i], "ab_params": ab2d} for i in range(NCORES)]
    res = run_bass_kernel_spmd(nc, in_maps, core_ids=list(range(NCORES)))
    return [res.results[i]["out"] for i in range(NCORES)], res


def kernel(x: np.ndarray, ab_params: np.ndarray) -> np.ndarray:
    assert x.shape == (B, D, H, W)
    xf = np.ascontiguousarray(x.reshape(B, D, N))
    xbf = xf.astype(ml_dtypes.bfloat16)
    ab2d = np.ascontiguousarray(ab_params.reshape(1, 3)).astype(np.float32)
    shards = [xbf[i * BPC:(i + 1) * BPC] for i in range(NCORES)]
    outs, _ = run_sharded(shards, ab2d)
    out = np.concatenate(outs, axis=0).astype(np.float32)
    return out.reshape(B, D, H, W)


# revision 4
# speedup vs baseline: 663.9337x; 663.9337x over previous
"""AntiBurst kernel for Trainium2 — data-parallel over batch on 8 NeuronCores.

Reference (per batch b, x: (B=32, D=768, H=32, W=32), N = H*W = 1024):
    xf  = x[b].reshape(D, N)
    G   = xf.T @ xf
    dis = (-2 + 2*G)*ab_w + ab_b
    w   = sum_m sigmoid(dis[:, m])
    out = xf / w**ab_p

Device kernel (per core, 4 batches):
  - Gram matrix via TensorE in bf16 (host pre-casts x), f32 PSUM accum.
    Only the upper-triangular block strips are computed (G is symmetric):
    row-block j covers columns j*128..N. 56% of the full matmul work.
  - sum_m sigmoid(z) = N/2 + 0.5*sum_m tanh(z/2): tanh + row-sum fused in one
    ScalarE activation (accum_out), scale/bias from runtime ab_params.
    Tanh and Exp share one activation-table set, so the table loads once.
  - The missing lower-triangle row-sum contributions are column sums of the
    strips (symmetry): a running strip-sum S is accumulated on VectorE, then
    one ones-vector matmul per block produces the column sums in [128,1]
    layout directly.
  - w^-p = exp(-p*ln(w)): log2(w) computed on VectorE with exponent/mantissa
    bit tricks (deg-3 polynomial), Exp on ScalarE. No extra table set.
  - r = w^-p is transposed with VectorE 32x32 block transposes, gathered to a
    single row via SBUF->SBUF DMA, broadcast to all 128 partitions with a
    K=1 ones-matmul on TensorE, and multiplied into x on VectorE (bf16).
"""
import numpy as np
import ml_dtypes

import concourse.bass as bass
import concourse.mybir as mybir
import concourse.tile as tile
from concourse import bacc
from concourse.bass_utils import run_bass_kernel_spmd

F32 = mybir.dt.float32
I32 = mybir.dt.int32
BF16 = mybir.dt.bfloat16
B, D, H, W = 32, 768, 32, 32
N = H * W
NCORES = 8
BPC = B // NCORES
DC = D // 128
NJ = N // 128

# deg-3 fit of log2(m) on [1,2), in ((m+A)*m + B)*m*C3 + C0 form
_m = np.linspace(1.0, 2.0, 4097)
_C = np.polyfit(_m, np.log2(_m), 3)
C3, C2, C1, C0 = [float(c) for c in _C]
A_ = C2 / C3
B_ = C1 / C3
LN2 = float(np.log(2.0))


def build(iters: int = 1, dyn: int = 0):
    nc = bacc.Bacc("TRN2", target_bir_lowering=False, debug=False, num_devices=NCORES)
    x_d = nc.dram_tensor("x", [BPC, DC, 128, N], BF16, kind="ExternalInput").ap()
    ab_d = nc.dram_tensor("ab_params", [1, 3], F32, kind="ExternalInput").ap()
    out_d = nc.dram_tensor("out", [BPC, DC, 128, N], BF16, kind="ExternalOutput").ap()

    with tile.TileContext(nc) as tc:
        with (
            tc.tile_pool(name="const", bufs=1) as constp,
            tc.tile_pool(name="x", bufs=18) as xpool,
            tc.tile_pool(name="sig", bufs=4) as sigpool,
            tc.tile_pool(name="wsm", bufs=3) as wpool,
            tc.tile_pool(name="rbc", bufs=2) as rbcpool,
            tc.tile_pool(name="ob", bufs=4) as opool,
            tc.tile_pool(name="psg", bufs=2, space="PSUM") as psgpool,
            tc.tile_pool(name="psc", bufs=2, space="PSUM") as pscpool,
            tc.tile_pool(name="psb", bufs=2, space="PSUM") as psbpool,
            tc.tile_pool(name="S", bufs=2) as spool,
        ):
            # runtime scalars: tanh arg = (s*G + t)/2 with s = 2 ab_w,
            # t = ab_b - 2 ab_w  ->  scale = ab_w, bias = ab_b/2 - ab_w
            ab_sb = constp.tile([128, 3], F32)
            nc.sync.dma_start(ab_sb[:], ab_d.to_broadcast((128, 3)))
            s_half = constp.tile([128, 1], F32)
            nc.vector.tensor_copy(s_half[:], ab_sb[:, 0:1])
            t_half = constp.tile([128, 1], F32)
            nc.vector.tensor_scalar(t_half[:], ab_sb[:, 1:2], 0.5, None,
                                    mybir.AluOpType.mult)
            nc.vector.tensor_sub(t_half[:], t_half[:], ab_sb[:, 0:1])
            negp_ln2 = constp.tile([128, 1], F32)
            nc.vector.tensor_scalar_mul(negp_ln2[:], ab_sb[:, 2:3], -LN2)
            ones_col = constp.tile([128, 1], BF16)
            nc.vector.memset(ones_col[:], 1.0)
            ones_bf_row = constp.tile([1, 128], BF16)
            nc.vector.memset(ones_bf_row[:], 1.0)

            def emit_loads(b):
                xc = []
                for c in range(DC):
                    t = xpool.tile([128, N], BF16, tag="x")
                    nc.sync.dma_start(t[:], x_d[b, c])
                    xc.append(t)
                return xc

            def emit_strip(b, j, xc, acc):
                wdt = N - j * 128
                psG = psgpool.tile([128, N], F32, tag="psG")
                for c in range(DC):
                    lhsT = xc[c][:, j * 128:(j + 1) * 128]
                    o = 0
                    while o < wdt:
                        nn = min(512, wdt - o)
                        nc.tensor.matmul(
                            psG[:, o:o + nn],
                            lhsT,
                            xc[c][:, j * 128 + o:j * 128 + o + nn],
                            start=(c == 0),
                            stop=(c == DC - 1),
                        )
                        o += nn
                sg = sigpool.tile([128, N], BF16, tag="sg")
                nc.scalar.activation(
                    sg[:, 0:wdt], psG[:, 0:wdt],
                    mybir.ActivationFunctionType.Tanh,
                    bias=t_half[:], scale=s_half[:],
                    accum_out=acc[:, j:j + 1],
                )
                return sg

            def emit_saccum(j, sg, S):
                if j == 0:
                    nc.vector.tensor_copy(S[:], sg[:, 128:N])
                elif j < NJ - 1:
                    nc.vector.tensor_add(
                        S[:, j * 128:N - 128],
                        S[:, j * 128:N - 128],
                        sg[:, 128:(NJ - j) * 128],
                    )

            def emit_colsums(S, wcols):
                # single-shot column sums; each column written exactly once
                # (start=True clears has_written for the whole bank, so no
                # accumulation groups may interleave in this bank)
                for mb in range(1, NJ):
                    nc.tensor.matmul(
                        wcols[:, mb:mb + 1],
                        S[:, (mb - 1) * 128:mb * 128],
                        ones_col[:],
                        start=True, stop=True, skip_group_check=True,
                    )

            def emit_tail(b, xc, acc, wcols):
                # w2 = 2*w = acc + wcols + N ; log2(w) = log2(w2) - 1
                tot = wpool.tile([128, NJ], F32, tag="tot")
                nc.vector.tensor_copy(tot[:, 0:1], acc[:, 0:1])
                nc.vector.tensor_add(tot[:, 1:NJ], acc[:, 1:NJ], wcols[:, 1:NJ])
                w2 = wpool.tile([128, NJ], F32, tag="w2")
                nc.vector.tensor_scalar_add(w2[:], tot[:], float(N))
                iw = w2[:].bitcast(I32)
                e_i = wpool.tile([128, NJ], I32, tag="ei")
                nc.vector.tensor_scalar(
                    e_i[:], iw, 23, None, mybir.AluOpType.arith_shift_right
                )
                e_f = wpool.tile([128, NJ], F32, tag="ef")
                nc.vector.tensor_copy(e_f[:], e_i[:])
                m_i = wpool.tile([128, NJ], I32, tag="mi")
                nc.vector.tensor_scalar(
                    m_i[:], iw, 0x007FFFFF, 0x3F800000,
                    mybir.AluOpType.bitwise_and, mybir.AluOpType.bitwise_or,
                )
                m_f = m_i[:].bitcast(F32)
                u = wpool.tile([128, NJ], F32, tag="u")
                nc.vector.scalar_tensor_tensor(
                    u[:], m_f, A_, m_f,
                    op0=mybir.AluOpType.add, op1=mybir.AluOpType.mult,
                )
                v = wpool.tile([128, NJ], F32, tag="v")
                nc.vector.scalar_tensor_tensor(
                    v[:], u[:], B_, m_f,
                    op0=mybir.AluOpType.add, op1=mybir.AluOpType.mult,
                )
                pre = wpool.tile([128, NJ], F32, tag="pre")
                nc.vector.tensor_scalar(
                    pre[:], v[:], C3, C0 - 128.0,
                    mybir.AluOpType.mult, mybir.AluOpType.add,
                )
                lg = wpool.tile([128, NJ], F32, tag="lg")
                nc.vector.tensor_add(lg[:], pre[:], e_f[:])

                # r = exp(-p*ln2*lg) = w^-p
                r_pad = wpool.tile([128, 32], BF16, tag="rpad")
                nc.vector.memset(r_pad[:, NJ:32], 0.0)
                nc.scalar.activation(
                    r_pad[:, 0:NJ], lg[:], mybir.ActivationFunctionType.Exp,
                    scale=negp_ln2[:],
                )
                r_t = wpool.tile([32, 128], BF16, tag="rt")
                for q in range(4):
                    nc.vector.transpose(
                        r_t[0:32, q * 32:(q + 1) * 32],
                        r_pad[q * 32:(q + 1) * 32, 0:32],
                    )
                r_row = wpool.tile([1, N], BF16, tag="rrow")
                nc.sync.dma_start(
                    r_row[:].rearrange("o (j p) -> o j p", p=128),
                    r_t[0:NJ, :],
                )
                r_bc = rbcpool.tile([128, N], BF16)
                for hh in range(2):
                    psB = psbpool.tile([128, 512], F32, tag="psB")
                    nc.tensor.matmul(
                        psB[:], ones_bf_row[:], r_row[:, hh * 512:(hh + 1) * 512],
                    )
                    nc.vector.tensor_copy(r_bc[:, hh * 512:(hh + 1) * 512], psB[:])
                for c in range(DC):
                    obf = opool.tile([128, N], BF16, tag="ob")
                    nc.vector.tensor_mul(obf[:], xc[c][:], r_bc[:])
                    nc.sync.dma_start(out_d[b, c], obf[:])

            def emit_iter():
                pending = None
                for b in range(BPC):
                    xc = emit_loads(b)
                    acc = wpool.tile([128, NJ], F32, tag="acc")
                    wcols = pscpool.tile([128, NJ], F32, tag="wcols")
                    S = spool.tile([128, N - 128], BF16, tag="S")
                    sg_prev = emit_strip(b, 0, xc, acc)
                    if pending is not None:
                        emit_tail(*pending)
                    for j in range(1, NJ):
                        sg_j = emit_strip(b, j, xc, acc)
                        emit_saccum(j - 1, sg_prev, S)
                        sg_prev = sg_j
                    emit_colsums(S, wcols)
                    pending = (b, xc, acc, wcols)
                emit_tail(*pending)

            if dyn:
                with tc.For_i(0, dyn, 1):
                    emit_iter()
            else:
                for _ in range(iters):
                    emit_iter()
    nc.compile()
    return nc


_CACHE: dict = {}


def _get_nc(iters: int = 1):
    if iters not in _CACHE:
        _CACHE[iters] = build(iters)
    return _CACHE[iters]


def run_sharded(xbf_shards, ab2d, iters: int = 1):
    nc = _get_nc(iters)
    in_maps = [{"x": xbf_shards[i], "ab_params": ab2d} for i in range(NCORES)]
    res = run_bass_kernel_spmd(nc, in_maps, core_ids=list(range(NCORES)))
    return [res.results[i]["out"] for i in range(NCORES)], res


def kernel(x: np.ndarray, ab_params: np.ndarray) -> np.ndarray:
    assert x.shape == (B, D, H, W), f"unexpected x shape {x.shape}"
    xf = np.ascontiguousarray(np.asarray(x, dtype=np.float32).reshape(B, DC, 128, N))
    xbf = xf.astype(ml_dtypes.bfloat16)
    ab2d = np.ascontiguousarray(
        np.asarray(ab_params, dtype=np.float32).reshape(1, 3))
    shards = [xbf[i * BPC:(i + 1) * BPC] for i in range(NCORES)]
    outs, _ = run_sharded(shards, ab2d)
    out = np.concatenate(outs, axis=0).astype(np.float32)
    return out.reshape(B, D, H, W)
